# revision 15
# baseline (speedup 1.0000x reference)
"""GATv2Conv (PyG-style, concat=False) forward on 8 Trainium2 NeuronCores.

Strategy (dst-sharded message passing):
  - Each core owns a contiguous range of 6250 destination nodes (49 blocks of
    128 dst slots).  All edges whose dst falls in the core's range are
    processed there, so the softmax over incoming edges is core-local (no
    collectives).
  - Host sorts each core's edges by (dst block, src-table-half), pads each
    (block, half) group to a multiple of 128 edges, and equalizes group tile
    counts across cores so one SPMD program fits all 8 cores.
  - Every core builds the full source-projection table
        xl~[n] = |att| * (x @ W_l)[n]           (bf16, [50176, 256] in HBM)
    and gathers per-edge rows with the SWDGE dma_gather (int16 indices, hence
    the A/B table-half split at row 32768).
  - Attention logits use the identity  att_c*leaky(m_c) = sign(att_c) *
    leaky(|att_c|*m_c), so all linear weights are pre-scaled by |att| on the
    host and the per-edge dot needs only Lrelu (ACT) + sign-mult + reduce
    (DVE).  exp() is applied with no segment-max (mathematically identical
    softmax; fp32 range is ample for this data distribution).
  - Per 128-edge tile, e = ea@W_e (+ biases) and x_r[dst] enter PSUM via
    matmuls (x_r via a host-sent one-hot S^T), gathered xl~ is injected with
    an identity matmul, and the weighted scatter into the 128-dst block is a
    single matmul with the host-sent one-hot S; PSUM accumulates the whole
    block including the softmax denominators as 4 extra columns.
  - Flush per block: out = mean_h( acc * 0.25/|att| * 1/(denom) ) + bias',
    where bias' = bias + mean_h(b_l) folds the (table-less) b_l back in.
"""

import math
import sys

import numpy as np

sys.path.insert(0, "/opt/trn_rl_repo")
sys.path.insert(0, "/opt/trn_rl_repo/concourse")

import ml_dtypes

N = 50000
E = 500000
IN_C = 128
HEADS = 4
OUT_C = 64
HC = HEADS * OUT_C  # 256
ED = 32
NEG = 0.2

NCORES = 8
NODES_PER_CORE = N // NCORES  # 6250
NBLK = 49  # ceil(6250/128)
BLK = 128
NPAD_TABLE = 50176  # 392 * 128
SPLIT = 32768  # A table rows [0,SPLIT), B table rows [SPLIT, NPAD_TABLE)
GB = 16  # tiles per gather call / per stream group
EXPB = 8  # tiles per exp batch

BF16 = ml_dtypes.bfloat16

_prog_cache = {}


# --------------------------------------------------------------------------
# Host preprocessing
# --------------------------------------------------------------------------
def _host_prep(x, edge_index, edge_attr, W_l, b_l, W_r, b_r, W_e, att, bias):
    att_f = att.reshape(HC).astype(np.float64)
    aabs = np.abs(att_f)
    sgn = np.sign(att_f)

    Wl_s = (W_l.astype(np.float64) * aabs).astype(BF16)  # [128, 256]
    Wr_s = (W_r.astype(np.float64) * aabs).astype(BF16)
    We_s = (W_e.astype(np.float64) * aabs).astype(np.float64)  # [32, 256]
    brow = (b_l.astype(np.float64) + b_r.astype(np.float64)) * aabs  # [256]
    We_ext = np.concatenate([We_s, brow[None, :]], axis=0).astype(BF16)  # [33,256]

    inv_att = np.where(aabs > 0, 0.25 / np.maximum(aabs, 1e-300), 0.0)
    inv_att = inv_att.astype(np.float32)[None, :]  # [1, 256]
    biasp = (bias.astype(np.float64) + b_l.reshape(HEADS, OUT_C).mean(0)).astype(
        np.float32
    )[None, :]  # [1, 64]
    sgn_bf = sgn.astype(BF16)[None, :]  # [1, 256]

    xT = np.zeros((IN_C, NPAD_TABLE), dtype=BF16)
    xT[:, :N] = x.T.astype(BF16)

    src = np.asarray(edge_index[0], dtype=np.int64)
    dst = np.asarray(edge_index[1], dtype=np.int64)
    core_of = dst // NODES_PER_CORE

    # per-core edge sets grouped by (block, half)
    per_core = []
    counts = np.zeros((NCORES, NBLK, 2), dtype=np.int64)
    for k in range(NCORES):
        sel = np.nonzero(core_of == k)[0]
        s_e = src[sel]
        d_loc = dst[sel] - k * NODES_PER_CORE
        blk = d_loc // BLK
        half = (s_e >= SPLIT).astype(np.int64)
        order = np.lexsort((half, blk))
        sel, s_e, d_loc, blk, half = (
            sel[order],
            s_e[order],
            d_loc[order],
            blk[order],
            half[order],
        )
        for b in range(NBLK):
            m = blk == b
            counts[k, b, 0] = np.sum(m & (half == 0))
            counts[k, b, 1] = np.sum(m & (half == 1))
        per_core.append((sel, s_e, d_loc, blk, half))

    ktiles = np.maximum(
        1, (counts.max(axis=0) + BLK - 1) // BLK
    )  # broadcast: max over cores -> ceil tiles; shape [NBLK, 2]
    # ensure at least one tile per block handled by the max(1, ...) on A side:
    kA = np.maximum(ktiles[:, 0], 1)
    kB = ktiles[:, 1].copy()
    kB[counts.max(axis=0)[:, 1] == 0] = 0
    T_total = int(np.sum(kA) + np.sum(kB))
    TA = int(np.sum(kA))
    TB = int(np.sum(kB))
    NG = (T_total + GB - 1) // GB
    CA = (TA + GB - 1) // GB  # A gather calls
    CB = (TB + GB - 1) // GB if TB > 0 else 0

    sched = {
        "kA": kA.tolist(),
        "kB": kB.tolist(),
        "T": T_total,
        "TA": TA,
        "TB": TB,
        "NG": NG,
        "CA": CA,
        "CB": CB,
    }

    # build per-core device arrays
    in_maps = []
    hostmeta = []
    for k in range(NCORES):
        sel, s_e, d_loc, blk, half = per_core[k]
        n_edges = len(sel)

        # tile-major per-edge arrays (T_total tiles of 128 edges)
        idxs_A = np.zeros(TA * BLK, dtype=np.int16)
        idxs_B = np.zeros(max(TB, 1) * BLK, dtype=np.int16)
        eaT = np.zeros((T_total, ED + 1, BLK), dtype=BF16)
        eaT[:, ED, :] = 1.0
        Smat = np.zeros((T_total, BLK, BLK), dtype=BF16)

        t_idx = 0
        a_ord = 0
        b_ord = 0
        ptr = 0
        for b in range(NBLK):
            for hf in (0, 1):
                ntile = int(kA[b]) if hf == 0 else int(kB[b])
                cnt = int(counts[k, b, hf])
                # real edges for this (block, half) are contiguous at ptr
                eidx = np.arange(ptr, ptr + cnt)
                ptr += cnt
                for j in range(ntile):
                    lo = j * BLK
                    hi = min((j + 1) * BLK, cnt)
                    nreal = max(0, hi - lo)
                    if nreal > 0:
                        ee = eidx[lo : lo + nreal]
                        rows = (d_loc[ee] % BLK).astype(np.int64)
                        if hf == 0:
                            idxs_A[a_ord * BLK : a_ord * BLK + nreal] = s_e[ee].astype(
                                np.int16
                            )
                        else:
                            idxs_B[b_ord * BLK : b_ord * BLK + nreal] = (
                                s_e[ee] - SPLIT
                            ).astype(np.int16)
                        eaT[t_idx, :ED, :nreal] = (
                            edge_attr[sel[ee]].astype(BF16).T
                        )
                        Smat[t_idx, np.arange(nreal), rows] = 1.0
                    if hf == 0:
                        a_ord += 1
                    else:
                        b_ord += 1
                    t_idx += 1
        assert t_idx == T_total and a_ord == TA and b_ord == TB
        assert ptr == n_edges

        STmat = np.ascontiguousarray(np.transpose(Smat, (0, 2, 1)))

        def group_major(arr, P, W):
            # [T, P, W] -> [NG, P, GB*W] with zero pad
            Tpad = NG * GB
            out = np.zeros((Tpad, P, W), dtype=arr.dtype)
            out[: arr.shape[0]] = arr
            out = out.reshape(NG, GB, P, W).transpose(0, 2, 1, 3)
            return np.ascontiguousarray(out.reshape(NG, P, GB * W))

        eaT_g = group_major(eaT, ED + 1, BLK)
        S_g = group_major(Smat, BLK, BLK)
        ST_g = group_major(STmat, BLK, BLK)
        SST_g = np.concatenate([S_g, ST_g], axis=2)

        def wrap_idx(flat, ncalls):
            # [L] -> [128, ncalls*128] int16, 16-partition wrap per call
            out = np.zeros((128, max(ncalls, 1) * 128), dtype=np.int16)
            for g in range(ncalls):
                seg = flat[g * GB * BLK : (g + 1) * GB * BLK]
                n = len(seg)
                if n == 0:
                    continue
                segp = np.zeros(GB * BLK, dtype=np.int16)
                segp[:n] = seg
                w16 = segp.reshape(-1, 16).T  # [16, 128]
                out[:, g * 128 : (g + 1) * 128] = np.tile(w16, (8, 1))
            return out

        idxA_w = wrap_idx(idxs_A, CA)
        idxB_w = wrap_idx(idxs_B, max(CB, 1))

        xTown = np.zeros((IN_C, NBLK * BLK), dtype=BF16)
        own = x[k * NODES_PER_CORE : (k + 1) * NODES_PER_CORE].T.astype(BF16)
        xTown[:, : own.shape[1]] = own

        in_maps.append(
            {
                "xT": xT,
                "xTown": xTown,
                "Wl": Wl_s,
                "Wr": Wr_s,
                "We": We_ext,
                "sgn": sgn_bf,
                "invatt": inv_att,
                "biasp": biasp,
                "eaT": eaT_g,
                "SST": SST_g,
                "idxA": idxA_w,
                "idxB": idxB_w,
            }
        )
        hostmeta.append({})
    return sched, in_maps


# --------------------------------------------------------------------------
# Bass program
# --------------------------------------------------------------------------
def _build_program(sched):
    import concourse.bass as bass
    import concourse.mybir as mybir
    import concourse.tile as tile
    from concourse import bacc, library_config
    from concourse.masks import make_identity

    f32 = mybir.dt.float32
    bf16 = mybir.dt.bfloat16
    i16 = mybir.dt.int16
    AF = mybir.ActivationFunctionType
    ALU = mybir.AluOpType
    AX = mybir.AxisListType

    kA, kB = sched["kA"], sched["kB"]
    T_total, TA, TB = sched["T"], sched["TA"], sched["TB"]
    NG, CA, CB = sched["NG"], sched["CA"], sched["CB"]

    nc = bacc.Bacc("TRN2", target_bir_lowering=False, debug=False, num_devices=NCORES)

    d_xT = nc.dram_tensor("xT", [IN_C, NPAD_TABLE], bf16, kind="ExternalInput")
    d_xTown = nc.dram_tensor("xTown", [IN_C, NBLK * BLK], bf16, kind="ExternalInput")
    d_Wl = nc.dram_tensor("Wl", [IN_C, HC], bf16, kind="ExternalInput")
    d_Wr = nc.dram_tensor("Wr", [IN_C, HC], bf16, kind="ExternalInput")
    d_We = nc.dram_tensor("We", [ED + 1, HC], bf16, kind="ExternalInput")
    d_sgn = nc.dram_tensor("sgn", [1, HC], bf16, kind="ExternalInput")
    d_invatt = nc.dram_tensor("invatt", [1, HC], f32, kind="ExternalInput")
    d_biasp = nc.dram_tensor("biasp", [1, OUT_C], f32, kind="ExternalInput")
    d_eaT = nc.dram_tensor("eaT", [NG, ED + 1, GB * BLK], bf16, kind="ExternalInput")
    d_SST = nc.dram_tensor(
        "SST", [NG, BLK, 2 * GB * BLK], bf16, kind="ExternalInput"
    )
    d_idxA = nc.dram_tensor("idxA", [128, max(CA, 1) * 128], i16, kind="ExternalInput")
    d_idxB = nc.dram_tensor("idxB", [128, max(CB, 1) * 128], i16, kind="ExternalInput")
    d_out = nc.dram_tensor("out", [NBLK * BLK, OUT_C], f32, kind="ExternalOutput")
    d_tableA = nc.dram_tensor("xl_tableA", [SPLIT, HC], bf16)
    d_tableB = nc.dram_tensor("xl_tableB", [NPAD_TABLE - SPLIT, HC], bf16)

    with tile.TileContext(nc) as tc:
        nc.gpsimd.load_library(library_config.mlp)

        with tc.tile_pool(name="singles", bufs=1) as singles:
            sb_Wl = singles.tile([IN_C, HC], bf16, tag="wl")
            nc.sync.dma_start(out=sb_Wl, in_=d_Wl.ap())
            sb_Wr = singles.tile([IN_C, HC], bf16, tag="wr")
            nc.sync.dma_start(out=sb_Wr, in_=d_Wr.ap())
            sb_We = singles.tile([ED + 1, HC], bf16, tag="we")
            nc.sync.dma_start(out=sb_We, in_=d_We.ap())
            sb_sgn = singles.tile([128, HC], bf16, tag="sgn")
            nc.sync.dma_start(out=sb_sgn, in_=d_sgn.ap().to_broadcast((128, HC)))
            sb_invatt = singles.tile([128, HC], f32, tag="invatt")
            nc.sync.dma_start(
                out=sb_invatt, in_=d_invatt.ap().to_broadcast((128, HC))
            )
            sb_biasp = singles.tile([128, OUT_C], f32, tag="biasp")
            nc.sync.dma_start(
                out=sb_biasp, in_=d_biasp.ap().to_broadcast((128, OUT_C))
            )
            sb_ident = singles.tile([128, 128], bf16, tag="ident")
            make_identity(nc, sb_ident)
            sb_idxA = singles.tile([128, max(CA, 1) * 128], i16, tag="idxA")
            nc.sync.dma_start(out=sb_idxA, in_=d_idxA.ap())
            sb_idxB = singles.tile([128, max(CB, 1) * 128], i16, tag="idxB")
            nc.sync.dma_start(out=sb_idxB, in_=d_idxB.ap())
            sb_xr = singles.tile([BLK, NBLK * HC], bf16, tag="xr")

            # ---------------- phase 1: xl~ table + xr~ -----------------
            with (
                tc.tile_pool(name="p1", bufs=3) as p1,
                tc.tile_pool(name="p1ps", bufs=2, space="PSUM") as p1ps,
            ):
                xto = p1.tile([IN_C, NBLK * BLK], bf16, tag="xto")
                nc.sync.dma_start(out=xto, in_=d_xTown.ap())
                for gg in range(7):  # 49 blocks in groups of 7
                    psr = p1ps.tile([128, 7 * HC], f32, tag="ps", name="psr")
                    for u_ in range(7):
                        b = gg * 7 + u_
                        nc.tensor.matmul(
                            psr[:, u_ * HC : (u_ + 1) * HC],
                            lhsT=xto[:, b * BLK : (b + 1) * BLK],
                            rhs=sb_Wr,
                            start=(u_ % 2 == 0),
                            stop=(u_ % 2 == 1) or (u_ == 6),
                        )
                    nc.vector.tensor_copy(
                        sb_xr[:, gg * 7 * HC : (gg + 1) * 7 * HC], psr
                    )

                ntile_tb = NPAD_TABLE // BLK  # 392
                TBG = 8
                for g in range(ntile_tb // TBG):  # 49 groups of 1024 nodes
                    xt = p1.tile([IN_C, TBG * BLK], bf16, tag="xt")
                    nc.scalar.dma_start(
                        out=xt,
                        in_=d_xT.ap()[:, g * TBG * BLK : (g + 1) * TBG * BLK],
                    )
                    ps = p1ps.tile([128, TBG * HC], f32, tag="ps")
                    for t in range(TBG):
                        nc.tensor.matmul(
                            ps[:, t * HC : (t + 1) * HC],
                            lhsT=xt[:, t * BLK : (t + 1) * BLK],
                            rhs=sb_Wl,
                            start=(t % 2 == 0),
                            stop=(t % 2 == 1),
                        )
                    tb = p1.tile([128, TBG * HC], bf16, tag="tb")
                    if g % 2 == 0:
                        nc.vector.tensor_copy(tb, ps)
                    else:
                        nc.scalar.copy(tb, ps)
                    if g < SPLIT // (TBG * BLK):
                        tgt, off = d_tableA, g * TBG * BLK * HC
                    else:
                        tgt, off = d_tableB, (g - SPLIT // (TBG * BLK)) * TBG * BLK * HC
                    out_ap = bass.AP(
                        tensor=tgt,
                        offset=off,
                        ap=[[HC, 128], [BLK * HC, TBG], [1, HC]],
                    )
                    in_ap = bass.AP(
                        tensor=tb.tensor,
                        offset=tb.offset,
                        ap=[tb.ap[0], [HC, TBG], [1, HC]],
                    )
                    nc.sync.dma_start(out=out_ap, in_=in_ap)

            # ---------------- phase 2: edge pipeline -----------------
            with (
                tc.tile_pool(name="gbuf", bufs=3) as pool_g,
                tc.tile_pool(name="stream", bufs=3) as pool_s,
                tc.tile_pool(name="msg", bufs=3) as pool_m,
                tc.tile_pool(name="work", bufs=6) as pool_w,
                tc.tile_pool(name="flush", bufs=2) as pool_f,
                tc.tile_pool(name="pm", bufs=6, space="PSUM") as pool_pm,
                tc.tile_pool(name="po", bufs=2, space="PSUM") as pool_po,
            ):
                tiles = []  # (block, half, first, last)
                for b in range(NBLK):
                    nb_t = kA[b] + kB[b]
                    c = 0
                    for j in range(kA[b]):
                        tiles.append((b, 0, c == 0, c == nb_t - 1))
                        c += 1
                    for j in range(kB[b]):
                        tiles.append((b, 1, c == 0, c == nb_t - 1))
                        c += 1
                a_ord = 0
                b_ord = 0
                gbufs_A = [None] * max(CA, 1)
                gbufs_B = [None] * max(CB, 1)
                grp = [None]
                po_by_block = {}

                def issue_gather(hf, ordn):
                    call = ordn // GB
                    bufs, idxsb, tbl = (
                        (gbufs_A, sb_idxA, d_tableA)
                        if hf == 0
                        else (gbufs_B, sb_idxB, d_tableB)
                    )
                    if bufs[call] is None:
                        tot = (TA if hf == 0 else TB) * BLK
                        n_idx = min(GB * BLK, tot - call * GB * BLK)
                        if hf == 0:
                            gb = pool_g.tile([128, GB, HC], bf16, tag="gA", name="gba")
                        else:
                            gb = pool_g.tile([128, GB, HC], bf16, tag="gB", name="gbb")
                        nc.gpsimd.dma_gather(
                            out_ap=gb[:, : n_idx // 128, :],
                            in_ap=tbl.ap(),
                            idxs_ap=idxsb[:, call * 128 : call * 128 + n_idx // 16],
                            num_idxs=n_idx,
                            num_idxs_reg=n_idx,
                            elem_size=HC,
                            single_packet=False,
                        )
                        bufs[call] = gb
                    return bufs[call], ordn % GB

                nbatch = (len(tiles) + EXPB - 1) // EXPB
                for bt in range(nbatch):
                    lo = bt * EXPB
                    hi = min((bt + 1) * EXPB, len(tiles))
                    nb = hi - lo
                    qb = pool_w.tile([128, EXPB * 4], f32, tag="q", name="qb")
                    mb = pool_m.tile([128, EXPB, 260], bf16, tag="msg", name="mbuf")
                    binfo = []
                    # ---- phase A per tile: stream loads, gathers, psum matmuls
                    for j in range(nb):
                        t = lo + j
                        b, hf, first, last = tiles[t]
                        g = t // GB
                        gslot = t % GB
                        if gslot == 0:
                            ea_sb = pool_s.tile([ED + 1, GB * BLK], bf16, tag="ea")
                            nc.scalar.dma_start(out=ea_sb, in_=d_eaT.ap()[g])
                            sst_sb = pool_s.tile(
                                [BLK, 2 * GB * BLK], bf16, tag="sst"
                            )
                            nc.scalar.dma_start(out=sst_sb, in_=d_SST.ap()[g])
                            grp[0] = (ea_sb, sst_sb)
                        ea_sb, sst_sb = grp[0]
                        if hf == 0:
                            gbt, slot = issue_gather(0, a_ord)
                            a_ord += 1
                        else:
                            gbt, slot = issue_gather(1, b_ord)
                            b_ord += 1
                        binfo.append(
                            (b, hf, first, last, gbt, slot, ea_sb, sst_sb, gslot)
                        )

                    # paired psum accumulation + logits
                    npair = (nb + 1) // 2
                    for p in range(npair):
                        j0 = 2 * p
                        pair = [j0] + ([j0 + 1] if j0 + 1 < nb else [])
                        pm = pool_pm.tile([128, 2 * HC], f32, tag="pm")
                        for jj in pair:
                            (b, hf, first, last, gbt, slot, ea_sb, sst_sb, gslot) = (
                                binfo[jj]
                            )
                            half = jj - j0
                            sl = pm[:, half * HC : (half + 1) * HC]
                            nc.tensor.matmul(
                                sl,
                                lhsT=ea_sb[:, gslot * BLK : (gslot + 1) * BLK],
                                rhs=sb_We,
                                start=(half == 0),
                                stop=False,
                            )
                            nc.tensor.matmul(
                                sl,
                                lhsT=sst_sb[
                                    :,
                                    GB * BLK
                                    + gslot * BLK : GB * BLK
                                    + (gslot + 1) * BLK,
                                ],
                                rhs=sb_xr[:, b * HC : (b + 1) * HC],
                                start=False,
                                stop=False,
                            )
                        # xl injection (merged when slots are adjacent)
                        j_a = binfo[pair[0]]
                        j_b = binfo[pair[-1]] if len(pair) == 2 else None
                        mergeable = (
                            j_b is not None
                            and j_a[4] is j_b[4]
                            and j_b[5] == j_a[5] + 1
                        )
                        if mergeable:
                            xl2 = j_a[4][:, j_a[5] : j_a[5] + 2, :]
                            nc.tensor.matmul(
                                pm, lhsT=sb_ident, rhs=xl2, start=False, stop=True
                            )
                        else:
                            for jj in pair:
                                info = binfo[jj]
                                half = jj - j0
                                nc.tensor.matmul(
                                    pm[:, half * HC : (half + 1) * HC],
                                    lhsT=sb_ident,
                                    rhs=info[4][:, info[5], :],
                                    start=False,
                                    stop=(jj == pair[-1]),
                                )
                        w_pair = HC * len(pair)
                        lk = pool_w.tile([128, 2 * HC], bf16, tag="lk")
                        nc.scalar.activation(
                            lk[:, :w_pair],
                            pm[:, :w_pair],
                            AF.Prelu,
                            bias=0.0,
                            scale=1.0,
                            alpha=NEG,
                        )
                        if p % 2 == 0:
                            uq = pool_w.tile([128, 4 * HC], bf16, tag="uq", name="uq")
                        sgn2 = bass.AP(
                            tensor=sb_sgn.tensor,
                            offset=sb_sgn.offset,
                            ap=[sb_sgn.ap[0], [0, len(pair)], [1, HC]],
                        )
                        nc.vector.tensor_tensor(
                            out=uq[:, (p % 2) * 2 * HC : (p % 2) * 2 * HC + w_pair],
                            in0=lk[:, :w_pair],
                            in1=sgn2,
                            op=ALU.mult,
                        )
                        quad_done = (p % 2 == 1) or (p == npair - 1)
                        if quad_done:
                            qlo = (p // 2) * 4  # first tile idx in quad
                            nseg = min(nb - qlo, 4) * 4
                            u_v = bass.AP(
                                tensor=uq.tensor,
                                offset=uq.offset,
                                ap=[uq.ap[0], [OUT_C, nseg], [1, OUT_C]],
                            )
                            nc.vector.tensor_reduce(
                                out=qb[:, qlo * 4 : qlo * 4 + nseg],
                                in_=u_v,
                                axis=AX.X,
                                op=ALU.add,
                            )

                    # ---- exp for the batch ----
                    w_ap = bass.AP(
                        tensor=mb.tensor,
                        offset=mb.offset + 256,
                        ap=[mb.ap[0], [260, nb], [1, 4]],
                    )
                    nc.scalar.activation(
                        w_ap, qb[:, : nb * 4], AF.Exp, bias=0.0, scale=1.0
                    )

                    # ---- phase B: messages (gpsimd) + scatter + flush ----
                    j = 0
                    while j < nb:
                        info0 = binfo[j]
                        info1 = binfo[j + 1] if j + 1 < nb else None
                        mergeable = (
                            info1 is not None
                            and info0[4] is info1[4]
                            and info1[5] == info0[5] + 1
                        )
                        if mergeable:
                            xl2 = info0[4][:, info0[5] : info0[5] + 2, :]
                            out2 = mb[:, j : j + 2, 0:256]
                            wrep2 = bass.AP(
                                tensor=mb.tensor,
                                offset=mb.offset + j * 260 + 256,
                                ap=[mb.ap[0], [260, 2], [1, 4], [0, OUT_C]],
                            )
                            nc.gpsimd.tensor_tensor(
                                out=out2, in0=xl2, in1=wrep2, op=ALU.mult
                            )
                            j += 2
                        else:
                            wrep = bass.AP(
                                tensor=mb.tensor,
                                offset=mb.offset + j * 260 + 256,
                                ap=[mb.ap[0], [1, 4], [0, OUT_C]],
                            )
                            nc.gpsimd.tensor_tensor(
                                out=mb[:, j, 0:256],
                                in0=info0[4][:, info0[5], :],
                                in1=wrep,
                                op=ALU.mult,
                            )
                            j += 1
                    for j in range(nb):
                        (b, hf, first, last, gbt, slot, ea_sb, sst_sb, gslot) = (
                            binfo[j]
                        )
                        if first:
                            po_by_block[b] = pool_po.tile(
                                [128, 260], f32, tag="po", name="po"
                            )
                        cur_po = po_by_block[b]
                        nc.tensor.matmul(
                            cur_po,
                            lhsT=sst_sb[:, gslot * BLK : (gslot + 1) * BLK],
                            rhs=mb[:, j, 0:260],
                            start=first,
                            stop=last,
                        )
                        if last:
                            den = pool_f.tile([128, 4], f32, tag="den")
                            nc.vector.tensor_scalar(
                                out=den,
                                in0=cur_po[:, 256:260],
                                scalar1=1e-30,
                                scalar2=None,
                                op0=ALU.add,
                            )
                            rec = pool_f.tile([128, 4], f32, tag="rec")
                            nc.vector.reciprocal(rec, den)
                            t2 = pool_f.tile([128, HC], f32, tag="t2")
                            nc.vector.tensor_tensor(
                                out=t2,
                                in0=cur_po[:, 0:256],
                                in1=sb_invatt,
                                op=ALU.mult,
                            )
                            t3 = pool_f.tile([128, HC], f32, tag="t3")
                            rrep = bass.AP(
                                tensor=rec.tensor,
                                offset=rec.offset,
                                ap=[rec.ap[0], [1, 4], [0, OUT_C]],
                            )
                            nc.vector.tensor_tensor(
                                out=t3, in0=t2, in1=rrep, op=ALU.mult
                            )
                            t3_v = bass.AP(
                                tensor=t3.tensor,
                                offset=t3.offset,
                                ap=[t3.ap[0], [1, OUT_C], [OUT_C, HEADS]],
                            )
                            osb = pool_f.tile([128, OUT_C], f32, tag="osb")
                            nc.vector.tensor_reduce(
                                out=osb, in_=t3_v, axis=AX.X, op=ALU.add
                            )
                            nc.vector.tensor_tensor(
                                out=osb, in0=osb, in1=sb_biasp, op=ALU.add
                            )
                            nc.sync.dma_start(
                                out=d_out.ap()[b * BLK : (b + 1) * BLK, :], in_=osb
                            )
                            del po_by_block[b]

    nc.compile()
    return nc


# --------------------------------------------------------------------------
# Entry point
# --------------------------------------------------------------------------
def kernel(
    x,
    edge_index,
    edge_attr,
    W_l,
    b_l,
    W_r,
    b_r,
    W_e,
    att,
    bias,
    _return_extras=False,
    **run_kwargs,
):
    from concourse import bass_utils

    x = np.asarray(x, dtype=np.float32)
    edge_index = np.asarray(edge_index)
    edge_attr = np.asarray(edge_attr, dtype=np.float32)

    sched, in_maps = _host_prep(
        x, edge_index, edge_attr, W_l, b_l, W_r, b_r, W_e, att, bias
    )

    key = (sched["T"], tuple(sched["kA"]), tuple(sched["kB"]))
    if key not in _prog_cache:
        _prog_cache[key] = _build_program(sched)
    nc = _prog_cache[key]

    res = bass_utils.run_bass_kernel_spmd(
        nc, in_maps, core_ids=list(range(NCORES)), **run_kwargs
    )
    out = np.empty((N, OUT_C), dtype=np.float32)
    for k in range(NCORES):
        out[k * NODES_PER_CORE : (k + 1) * NODES_PER_CORE] = res.results[k]["out"][
            :NODES_PER_CORE
        ]
    if _return_extras:
        return out, res
    return out


# revision 16
# speedup vs baseline: 3.9265x; 3.9265x over previous
"""GATv2Conv (PyG-style, concat=False) forward on 8 Trainium2 NeuronCores.

Strategy (dst-sharded message passing):
  - Each core owns a contiguous range of 6250 destination nodes (49 blocks of
    128 dst slots).  All edges whose dst falls in the core's range are
    processed there, so the softmax over incoming edges is core-local (no
    collectives).
  - Host sorts each core's edges by (dst block, src-table-half), pads each
    (block, half) group to a multiple of 128 edges, and equalizes group tile
    counts across cores so one SPMD program fits all 8 cores.
  - Every core builds the full source-projection table
        xl~[n] = |att| * (x @ W_l)[n]           (bf16, [50176, 256] in HBM)
    and gathers per-edge rows with the SWDGE dma_gather (int16 indices, hence
    the A/B table-half split at row 32768).
  - Attention logits use the identity  att_c*leaky(m_c) = sign(att_c) *
    leaky(|att_c|*m_c), so all linear weights are pre-scaled by |att| on the
    host and the per-edge dot needs only Lrelu (ACT) + sign-mult + reduce
    (DVE).  exp() is applied with no segment-max (mathematically identical
    softmax; fp32 range is ample for this data distribution).
  - Per 128-edge tile, e = ea@W_e (+ biases) and x_r[dst] enter PSUM via
    matmuls (x_r via a host-sent one-hot S^T), gathered xl~ is injected with
    an identity matmul, and the weighted scatter into the 128-dst block is a
    single matmul with the host-sent one-hot S; PSUM accumulates the whole
    block including the softmax denominators as 4 extra columns.
  - Flush per block: out = mean_h( acc * 0.25/|att| * 1/(denom) ) + bias',
    where bias' = bias + mean_h(b_l) folds the (table-less) b_l back in.
"""

import math
import sys

import numpy as np

sys.path.insert(0, "/opt/trn_rl_repo")
sys.path.insert(0, "/opt/trn_rl_repo/concourse")

import ml_dtypes

N = 50000
E = 500000
IN_C = 128
HEADS = 4
OUT_C = 64
HC = HEADS * OUT_C  # 256
ED = 32
NEG = 0.2

NCORES = 8
NODES_PER_CORE = N // NCORES  # 6250
NBLK = 49  # ceil(6250/128)
BLK = 128
NPAD_TABLE = 50176  # 392 * 128
SPLIT = 32768  # A table rows [0,SPLIT), B table rows [SPLIT, NPAD_TABLE)
GB = 16  # tiles per gather call / per stream group
EXPB = 8  # tiles per exp batch

BF16 = ml_dtypes.bfloat16
FP8 = ml_dtypes.float8_e4m3
MSG_ENGINE = "pool"  # "pool" | "dve" | "split"

_prog_cache = {}


# --------------------------------------------------------------------------
# Host preprocessing
# --------------------------------------------------------------------------
def _host_prep(x, edge_index, edge_attr, W_l, b_l, W_r, b_r, W_e, att, bias):
    att_f = att.reshape(HC).astype(np.float64)
    aabs = np.abs(att_f)
    sgn = np.sign(att_f)

    Wl_s = (W_l.astype(np.float64) * aabs).astype(BF16)  # [128, 256]
    Wr_s = (W_r.astype(np.float64) * aabs).astype(BF16)
    We_s = (W_e.astype(np.float64) * aabs).astype(np.float64)  # [32, 256]
    brow = (b_l.astype(np.float64) + b_r.astype(np.float64)) * aabs  # [256]
    We_ext = np.concatenate([We_s, brow[None, :]], axis=0).astype(BF16)  # [33,256]

    inv_att = np.where(aabs > 0, 0.25 / np.maximum(aabs, 1e-300), 0.0)
    inv_att = inv_att.astype(np.float32)[None, :]  # [1, 256]
    biasp = (bias.astype(np.float64) + b_l.reshape(HEADS, OUT_C).mean(0)).astype(
        np.float32
    )[None, :]  # [1, 64]
    sgn_bf = sgn.astype(BF16)[None, :]  # [1, 256]

    xT = np.zeros((IN_C, NPAD_TABLE), dtype=BF16)
    xT[:, :N] = x.T.astype(BF16)

    src = np.asarray(edge_index[0], dtype=np.int64)
    dst = np.asarray(edge_index[1], dtype=np.int64)
    core_of = dst // NODES_PER_CORE

    # per-core edge sets grouped by (block, half)
    per_core = []
    counts = np.zeros((NCORES, NBLK, 2), dtype=np.int64)
    for k in range(NCORES):
        sel = np.nonzero(core_of == k)[0]
        s_e = src[sel]
        d_loc = dst[sel] - k * NODES_PER_CORE
        blk = d_loc // BLK
        half = (s_e >= SPLIT).astype(np.int64)
        order = np.lexsort((half, blk))
        sel, s_e, d_loc, blk, half = (
            sel[order],
            s_e[order],
            d_loc[order],
            blk[order],
            half[order],
        )
        for b in range(NBLK):
            m = blk == b
            counts[k, b, 0] = np.sum(m & (half == 0))
            counts[k, b, 1] = np.sum(m & (half == 1))
        per_core.append((sel, s_e, d_loc, blk, half))

    ktiles = np.maximum(
        1, (counts.max(axis=0) + BLK - 1) // BLK
    )  # broadcast: max over cores -> ceil tiles; shape [NBLK, 2]
    # ensure at least one tile per block handled by the max(1, ...) on A side:
    kA = np.maximum(ktiles[:, 0], 1)
    kB = ktiles[:, 1].copy()
    kB[counts.max(axis=0)[:, 1] == 0] = 0
    T_total = int(np.sum(kA) + np.sum(kB))
    TA = int(np.sum(kA))
    TB = int(np.sum(kB))
    NG = (T_total + GB - 1) // GB
    CA = (TA + GB - 1) // GB  # A gather calls
    CB = (TB + GB - 1) // GB if TB > 0 else 0

    sched = {
        "kA": kA.tolist(),
        "kB": kB.tolist(),
        "T": T_total,
        "TA": TA,
        "TB": TB,
        "NG": NG,
        "CA": CA,
        "CB": CB,
    }

    # build per-core device arrays
    in_maps = []
    hostmeta = []
    for k in range(NCORES):
        sel, s_e, d_loc, blk, half = per_core[k]
        n_edges = len(sel)

        # tile-major per-edge arrays (T_total tiles of 128 edges)
        idxs_A = np.zeros(TA * BLK, dtype=np.int16)
        idxs_B = np.zeros(max(TB, 1) * BLK, dtype=np.int16)
        eaT = np.zeros((T_total, ED + 1, BLK), dtype=BF16)
        eaT[:, ED, :] = 1.0
        Smat = np.zeros((T_total, BLK, BLK), dtype=BF16)

        t_idx = 0
        a_ord = 0
        b_ord = 0
        ptr = 0
        for b in range(NBLK):
            for hf in (0, 1):
                ntile = int(kA[b]) if hf == 0 else int(kB[b])
                cnt = int(counts[k, b, hf])
                # real edges for this (block, half) are contiguous at ptr
                eidx = np.arange(ptr, ptr + cnt)
                ptr += cnt
                for j in range(ntile):
                    lo = j * BLK
                    hi = min((j + 1) * BLK, cnt)
                    nreal = max(0, hi - lo)
                    if nreal > 0:
                        ee = eidx[lo : lo + nreal]
                        rows = (d_loc[ee] % BLK).astype(np.int64)
                        if hf == 0:
                            idxs_A[a_ord * BLK : a_ord * BLK + nreal] = s_e[ee].astype(
                                np.int16
                            )
                        else:
                            idxs_B[b_ord * BLK : b_ord * BLK + nreal] = (
                                s_e[ee] - SPLIT
                            ).astype(np.int16)
                        eaT[t_idx, :ED, :nreal] = (
                            edge_attr[sel[ee]].astype(BF16).T
                        )
                        Smat[t_idx, np.arange(nreal), rows] = 1.0
                    if hf == 0:
                        a_ord += 1
                    else:
                        b_ord += 1
                    t_idx += 1
        assert t_idx == T_total and a_ord == TA and b_ord == TB
        assert ptr == n_edges

        STmat = np.ascontiguousarray(np.transpose(Smat, (0, 2, 1)))

        def group_major(arr, P, W):
            # [T, P, W] -> [NG, P, GB*W] with zero pad
            Tpad = NG * GB
            out = np.zeros((Tpad, P, W), dtype=arr.dtype)
            out[: arr.shape[0]] = arr
            out = out.reshape(NG, GB, P, W).transpose(0, 2, 1, 3)
            return np.ascontiguousarray(out.reshape(NG, P, GB * W))

        eaT_g = group_major(eaT, ED + 1, BLK)
        S_g = group_major(Smat, BLK, BLK)
        ST_g = group_major(STmat, BLK, BLK)
        SST_g = np.concatenate([S_g, ST_g], axis=2).astype(FP8)

        def wrap_idx(flat, ncalls):
            # [L] -> [128, ncalls*128] int16, 16-partition wrap per call
            out = np.zeros((128, max(ncalls, 1) * 128), dtype=np.int16)
            for g in range(ncalls):
                seg = flat[g * GB * BLK : (g + 1) * GB * BLK]
                n = len(seg)
                if n == 0:
                    continue
                segp = np.zeros(GB * BLK, dtype=np.int16)
                segp[:n] = seg
                w16 = segp.reshape(-1, 16).T  # [16, 128]
                out[:, g * 128 : (g + 1) * 128] = np.tile(w16, (8, 1))
            return out

        idxA_w = wrap_idx(idxs_A, CA)
        idxB_w = wrap_idx(idxs_B, max(CB, 1))

        xTown = np.zeros((IN_C, NBLK * BLK), dtype=BF16)
        own = x[k * NODES_PER_CORE : (k + 1) * NODES_PER_CORE].T.astype(BF16)
        xTown[:, : own.shape[1]] = own

        in_maps.append(
            {
                "xT": xT,
                "xTown": xTown,
                "Wl": Wl_s,
                "Wr": Wr_s,
                "We": We_ext,
                "sgn": sgn_bf,
                "invatt": inv_att,
                "biasp": biasp,
                "eaT": eaT_g,
                "SST": SST_g,
                "idxA": idxA_w,
                "idxB": idxB_w,
            }
        )
        hostmeta.append({})
    return sched, in_maps


# --------------------------------------------------------------------------
# Bass program
# --------------------------------------------------------------------------
def _build_program(sched):
    import concourse.bass as bass
    import concourse.mybir as mybir
    import concourse.tile as tile
    from concourse import bacc, library_config
    from concourse.masks import make_identity

    f32 = mybir.dt.float32
    bf16 = mybir.dt.bfloat16
    i16 = mybir.dt.int16
    AF = mybir.ActivationFunctionType
    ALU = mybir.AluOpType
    AX = mybir.AxisListType

    kA, kB = sched["kA"], sched["kB"]
    T_total, TA, TB = sched["T"], sched["TA"], sched["TB"]
    NG, CA, CB = sched["NG"], sched["CA"], sched["CB"]

    nc = bacc.Bacc("TRN2", target_bir_lowering=False, debug=False, num_devices=NCORES)

    d_xT = nc.dram_tensor("xT", [IN_C, NPAD_TABLE], bf16, kind="ExternalInput")
    d_xTown = nc.dram_tensor("xTown", [IN_C, NBLK * BLK], bf16, kind="ExternalInput")
    d_Wl = nc.dram_tensor("Wl", [IN_C, HC], bf16, kind="ExternalInput")
    d_Wr = nc.dram_tensor("Wr", [IN_C, HC], bf16, kind="ExternalInput")
    d_We = nc.dram_tensor("We", [ED + 1, HC], bf16, kind="ExternalInput")
    d_sgn = nc.dram_tensor("sgn", [1, HC], bf16, kind="ExternalInput")
    d_invatt = nc.dram_tensor("invatt", [1, HC], f32, kind="ExternalInput")
    d_biasp = nc.dram_tensor("biasp", [1, OUT_C], f32, kind="ExternalInput")
    d_eaT = nc.dram_tensor("eaT", [NG, ED + 1, GB * BLK], bf16, kind="ExternalInput")
    fp8 = mybir.dt.float8e4
    d_SST = nc.dram_tensor(
        "SST", [NG, BLK, 2 * GB * BLK], fp8, kind="ExternalInput"
    )
    d_idxA = nc.dram_tensor("idxA", [128, max(CA, 1) * 128], i16, kind="ExternalInput")
    d_idxB = nc.dram_tensor("idxB", [128, max(CB, 1) * 128], i16, kind="ExternalInput")
    d_out = nc.dram_tensor("out", [NBLK * BLK, OUT_C], f32, kind="ExternalOutput")
    d_tableA = nc.dram_tensor("xl_tableA", [SPLIT, HC], bf16)
    d_tableB = nc.dram_tensor("xl_tableB", [NPAD_TABLE - SPLIT, HC], bf16)

    with tile.TileContext(nc) as tc:
        nc.gpsimd.load_library(library_config.mlp)

        with tc.tile_pool(name="singles", bufs=1) as singles:
            sb_Wl = singles.tile([IN_C, HC], bf16, tag="wl")
            nc.sync.dma_start(out=sb_Wl, in_=d_Wl.ap())
            sb_Wr = singles.tile([IN_C, HC], bf16, tag="wr")
            nc.sync.dma_start(out=sb_Wr, in_=d_Wr.ap())
            sb_We = singles.tile([ED + 1, HC], bf16, tag="we")
            nc.sync.dma_start(out=sb_We, in_=d_We.ap())
            sb_sgn = singles.tile([128, HC], bf16, tag="sgn")
            nc.sync.dma_start(out=sb_sgn, in_=d_sgn.ap().to_broadcast((128, HC)))
            sb_invatt = singles.tile([128, HC], f32, tag="invatt")
            nc.sync.dma_start(
                out=sb_invatt, in_=d_invatt.ap().to_broadcast((128, HC))
            )
            sb_biasp = singles.tile([128, OUT_C], f32, tag="biasp")
            nc.sync.dma_start(
                out=sb_biasp, in_=d_biasp.ap().to_broadcast((128, OUT_C))
            )
            sb_ident = singles.tile([128, 128], bf16, tag="ident")
            make_identity(nc, sb_ident)
            sb_idxA = singles.tile([128, max(CA, 1) * 128], i16, tag="idxA")
            nc.sync.dma_start(out=sb_idxA, in_=d_idxA.ap())
            sb_idxB = singles.tile([128, max(CB, 1) * 128], i16, tag="idxB")
            nc.sync.dma_start(out=sb_idxB, in_=d_idxB.ap())
            sb_xr = singles.tile([BLK, NBLK * HC], bf16, tag="xr")

            # ---------------- phase 1: xl~ table + xr~ -----------------
            with (
                tc.tile_pool(name="p1", bufs=3) as p1,
                tc.tile_pool(name="p1ps", bufs=2, space="PSUM") as p1ps,
            ):
                xto = p1.tile([IN_C, NBLK * BLK], bf16, tag="xto")
                nc.sync.dma_start(out=xto, in_=d_xTown.ap())
                for gg in range(7):  # 49 blocks in groups of 7
                    psr = p1ps.tile([128, 7 * HC], f32, tag="ps", name="psr")
                    for u_ in range(7):
                        b = gg * 7 + u_
                        nc.tensor.matmul(
                            psr[:, u_ * HC : (u_ + 1) * HC],
                            lhsT=xto[:, b * BLK : (b + 1) * BLK],
                            rhs=sb_Wr,
                            start=(u_ % 2 == 0),
                            stop=(u_ % 2 == 1) or (u_ == 6),
                        )
                    nc.vector.tensor_copy(
                        sb_xr[:, gg * 7 * HC : (gg + 1) * 7 * HC], psr
                    )

                ntile_tb = NPAD_TABLE // BLK  # 392
                TBG = 8
                for g in range(ntile_tb // TBG):  # 49 groups of 1024 nodes
                    xt = p1.tile([IN_C, TBG * BLK], bf16, tag="xt")
                    nc.scalar.dma_start(
                        out=xt,
                        in_=d_xT.ap()[:, g * TBG * BLK : (g + 1) * TBG * BLK],
                    )
                    ps = p1ps.tile([128, TBG * HC], f32, tag="ps")
                    for t in range(TBG):
                        nc.tensor.matmul(
                            ps[:, t * HC : (t + 1) * HC],
                            lhsT=xt[:, t * BLK : (t + 1) * BLK],
                            rhs=sb_Wl,
                            start=(t % 2 == 0),
                            stop=(t % 2 == 1),
                        )
                    tb = p1.tile([128, TBG * HC], bf16, tag="tb")
                    if g % 2 == 0:
                        nc.vector.tensor_copy(tb, ps)
                    else:
                        nc.scalar.copy(tb, ps)
                    if g < SPLIT // (TBG * BLK):
                        tgt, off = d_tableA, g * TBG * BLK * HC
                    else:
                        tgt, off = d_tableB, (g - SPLIT // (TBG * BLK)) * TBG * BLK * HC
                    out_ap = bass.AP(
                        tensor=tgt,
                        offset=off,
                        ap=[[HC, 128], [BLK * HC, TBG], [1, HC]],
                    )
                    in_ap = bass.AP(
                        tensor=tb.tensor,
                        offset=tb.offset,
                        ap=[tb.ap[0], [HC, TBG], [1, HC]],
                    )
                    nc.sync.dma_start(out=out_ap, in_=in_ap)

            # ---------------- phase 2: edge pipeline -----------------
            with (
                tc.tile_pool(name="gbuf", bufs=4) as pool_g,
                tc.tile_pool(name="stream", bufs=3) as pool_s,
                tc.tile_pool(name="msg", bufs=3) as pool_m,
                tc.tile_pool(name="work", bufs=6) as pool_w,
                tc.tile_pool(name="flush", bufs=2) as pool_f,
                tc.tile_pool(name="pm", bufs=6, space="PSUM") as pool_pm,
                tc.tile_pool(name="po", bufs=2, space="PSUM") as pool_po,
            ):
                tiles = []  # (block, half, first, last)
                for b in range(NBLK):
                    nb_t = kA[b] + kB[b]
                    c = 0
                    for j in range(kA[b]):
                        tiles.append((b, 0, c == 0, c == nb_t - 1))
                        c += 1
                    for j in range(kB[b]):
                        tiles.append((b, 1, c == 0, c == nb_t - 1))
                        c += 1
                a_ord = 0
                b_ord = 0
                gbufs_A = [None] * max(CA, 1)
                gbufs_B = [None] * max(CB, 1)
                grp = [None]
                po_by_block = {}

                def issue_gather(hf, ordn):
                    call = ordn // GB
                    bufs, idxsb, tbl = (
                        (gbufs_A, sb_idxA, d_tableA)
                        if hf == 0
                        else (gbufs_B, sb_idxB, d_tableB)
                    )
                    if bufs[call] is None:
                        tot = (TA if hf == 0 else TB) * BLK
                        n_idx = min(GB * BLK, tot - call * GB * BLK)
                        if hf == 0:
                            gb = pool_g.tile([128, GB, HC], bf16, tag="gA", name="gba")
                        else:
                            gb = pool_g.tile([128, GB, HC], bf16, tag="gB", name="gbb")
                        nc.gpsimd.dma_gather(
                            out_ap=gb[:, : n_idx // 128, :],
                            in_ap=tbl.ap(),
                            idxs_ap=idxsb[:, call * 128 : call * 128 + n_idx // 16],
                            num_idxs=n_idx,
                            num_idxs_reg=n_idx,
                            elem_size=HC,
                            single_packet=False,
                        )
                        bufs[call] = gb
                    return bufs[call], ordn % GB

                nbatch = (len(tiles) + EXPB - 1) // EXPB
                for bt in range(nbatch):
                    lo = bt * EXPB
                    hi = min((bt + 1) * EXPB, len(tiles))
                    nb = hi - lo
                    qb = pool_w.tile([128, EXPB * 4], f32, tag="q", name="qb")
                    mb = pool_m.tile([128, EXPB, 260], bf16, tag="msg", name="mbuf")
                    binfo = []
                    # ---- phase A per tile: stream loads, gathers, psum matmuls
                    for j in range(nb):
                        t = lo + j
                        b, hf, first, last = tiles[t]
                        g = t // GB
                        gslot = t % GB
                        if gslot == 0:
                            ea_sb = pool_s.tile([ED + 1, GB * BLK], bf16, tag="ea")
                            nc.scalar.dma_start(out=ea_sb, in_=d_eaT.ap()[g])
                            sst_sb = pool_s.tile(
                                [BLK, 2 * GB * BLK], fp8, tag="sst"
                            )
                            nc.scalar.dma_start(out=sst_sb, in_=d_SST.ap()[g])
                            grp[0] = (ea_sb, sst_sb)
                        ea_sb, sst_sb = grp[0]
                        if hf == 0:
                            gbt, slot = issue_gather(0, a_ord)
                            a_ord += 1
                        else:
                            gbt, slot = issue_gather(1, b_ord)
                            b_ord += 1
                        binfo.append(
                            (b, hf, first, last, gbt, slot, ea_sb, sst_sb, gslot)
                        )

                    # paired psum accumulation + logits
                    npair = (nb + 1) // 2
                    for p in range(npair):
                        j0 = 2 * p
                        pair = [j0] + ([j0 + 1] if j0 + 1 < nb else [])
                        pm = pool_pm.tile([128, 2 * HC], f32, tag="pm")
                        for jj in pair:
                            (b, hf, first, last, gbt, slot, ea_sb, sst_sb, gslot) = (
                                binfo[jj]
                            )
                            half = jj - j0
                            sl = pm[:, half * HC : (half + 1) * HC]
                            nc.tensor.matmul(
                                sl,
                                lhsT=ea_sb[:, gslot * BLK : (gslot + 1) * BLK],
                                rhs=sb_We,
                                start=(half == 0),
                                stop=False,
                            )
                            nc.tensor.matmul(
                                sl,
                                lhsT=sst_sb[
                                    :,
                                    GB * BLK
                                    + gslot * BLK : GB * BLK
                                    + (gslot + 1) * BLK,
                                ],
                                rhs=sb_xr[:, b * HC : (b + 1) * HC],
                                start=False,
                                stop=False,
                            )
                        # xl injection (merged when slots are adjacent)
                        j_a = binfo[pair[0]]
                        j_b = binfo[pair[-1]] if len(pair) == 2 else None
                        mergeable = (
                            j_b is not None
                            and j_a[4] is j_b[4]
                            and j_b[5] == j_a[5] + 1
                        )
                        if mergeable:
                            xl2 = j_a[4][:, j_a[5] : j_a[5] + 2, :]
                            nc.tensor.matmul(
                                pm, lhsT=sb_ident, rhs=xl2, start=False, stop=True
                            )
                        else:
                            for jj in pair:
                                info = binfo[jj]
                                half = jj - j0
                                nc.tensor.matmul(
                                    pm[:, half * HC : (half + 1) * HC],
                                    lhsT=sb_ident,
                                    rhs=info[4][:, info[5], :],
                                    start=False,
                                    stop=(jj == pair[-1]),
                                )
                        w_pair = HC * len(pair)
                        lk = pool_w.tile([128, 2 * HC], bf16, tag="lk")
                        nc.scalar.activation(
                            lk[:, :w_pair],
                            pm[:, :w_pair],
                            AF.Prelu,
                            bias=0.0,
                            scale=1.0,
                            alpha=NEG,
                        )
                        if p % 2 == 0:
                            uq = pool_w.tile([128, 4 * HC], bf16, tag="uq", name="uq")
                        sgn2 = bass.AP(
                            tensor=sb_sgn.tensor,
                            offset=sb_sgn.offset,
                            ap=[sb_sgn.ap[0], [0, len(pair)], [1, HC]],
                        )
                        nc.vector.tensor_tensor(
                            out=uq[:, (p % 2) * 2 * HC : (p % 2) * 2 * HC + w_pair],
                            in0=lk[:, :w_pair],
                            in1=sgn2,
                            op=ALU.mult,
                        )
                        quad_done = (p % 2 == 1) or (p == npair - 1)
                        if quad_done:
                            qlo = (p // 2) * 4  # first tile idx in quad
                            nseg = min(nb - qlo, 4) * 4
                            u_v = bass.AP(
                                tensor=uq.tensor,
                                offset=uq.offset,
                                ap=[uq.ap[0], [OUT_C, nseg], [1, OUT_C]],
                            )
                            nc.vector.tensor_reduce(
                                out=qb[:, qlo * 4 : qlo * 4 + nseg],
                                in_=u_v,
                                axis=AX.X,
                                op=ALU.add,
                            )

                    # ---- exp for the batch ----
                    w_ap = bass.AP(
                        tensor=mb.tensor,
                        offset=mb.offset + 256,
                        ap=[mb.ap[0], [260, nb], [1, 4]],
                    )
                    nc.scalar.activation(
                        w_ap, qb[:, : nb * 4], AF.Exp, bias=0.0, scale=1.0
                    )

                    # ---- phase B: messages (gpsimd) + scatter + flush ----
                    j = 0
                    while j < nb:
                        info0 = binfo[j]
                        info1 = binfo[j + 1] if j + 1 < nb else None
                        mergeable = (
                            info1 is not None
                            and info0[4] is info1[4]
                            and info1[5] == info0[5] + 1
                        )
                        if mergeable:
                            xl2 = info0[4][:, info0[5] : info0[5] + 2, :]
                            out2 = mb[:, j : j + 2, 0:256]
                            wrep2 = bass.AP(
                                tensor=mb.tensor,
                                offset=mb.offset + j * 260 + 256,
                                ap=[mb.ap[0], [260, 2], [1, 4], [0, OUT_C]],
                            )
                            eng = (
                                nc.gpsimd
                                if MSG_ENGINE == "pool"
                                or (MSG_ENGINE == "split" and (j // 2) % 2 == 0)
                                else nc.vector
                            )
                            eng.tensor_tensor(
                                out=out2, in0=xl2, in1=wrep2, op=ALU.mult
                            )
                            j += 2
                        else:
                            wrep = bass.AP(
                                tensor=mb.tensor,
                                offset=mb.offset + j * 260 + 256,
                                ap=[mb.ap[0], [1, 4], [0, OUT_C]],
                            )
                            eng = (
                                nc.gpsimd
                                if MSG_ENGINE == "pool"
                                or (MSG_ENGINE == "split" and j % 2 == 0)
                                else nc.vector
                            )
                            eng.tensor_tensor(
                                out=mb[:, j, 0:256],
                                in0=info0[4][:, info0[5], :],
                                in1=wrep,
                                op=ALU.mult,
                            )
                            j += 1
                    for j in range(nb):
                        (b, hf, first, last, gbt, slot, ea_sb, sst_sb, gslot) = (
                            binfo[j]
                        )
                        if first:
                            po_by_block[b] = pool_po.tile(
                                [128, 260], f32, tag="po", name="po"
                            )
                        cur_po = po_by_block[b]
                        nc.tensor.matmul(
                            cur_po,
                            lhsT=sst_sb[:, gslot * BLK : (gslot + 1) * BLK],
                            rhs=mb[:, j, 0:260],
                            start=first,
                            stop=last,
                        )
                        if last:
                            den = pool_f.tile([128, 4], f32, tag="den")
                            nc.vector.tensor_scalar(
                                out=den,
                                in0=cur_po[:, 256:260],
                                scalar1=1e-30,
                                scalar2=None,
                                op0=ALU.add,
                            )
                            rec = pool_f.tile([128, 4], f32, tag="rec")
                            nc.vector.reciprocal(rec, den)
                            t2 = pool_f.tile([128, HC], f32, tag="t2")
                            nc.vector.tensor_tensor(
                                out=t2,
                                in0=cur_po[:, 0:256],
                                in1=sb_invatt,
                                op=ALU.mult,
                            )
                            t3 = pool_f.tile([128, HC], f32, tag="t3")
                            rrep = bass.AP(
                                tensor=rec.tensor,
                                offset=rec.offset,
                                ap=[rec.ap[0], [1, 4], [0, OUT_C]],
                            )
                            nc.vector.tensor_tensor(
                                out=t3, in0=t2, in1=rrep, op=ALU.mult
                            )
                            t3_v = bass.AP(
                                tensor=t3.tensor,
                                offset=t3.offset,
                                ap=[t3.ap[0], [1, OUT_C], [OUT_C, HEADS]],
                            )
                            osb = pool_f.tile([128, OUT_C], f32, tag="osb")
                            nc.vector.tensor_reduce(
                                out=osb, in_=t3_v, axis=AX.X, op=ALU.add
                            )
                            nc.vector.tensor_tensor(
                                out=osb, in0=osb, in1=sb_biasp, op=ALU.add
                            )
                            nc.sync.dma_start(
                                out=d_out.ap()[b * BLK : (b + 1) * BLK, :], in_=osb
                            )
                            del po_by_block[b]

    nc.compile()
    return nc


# --------------------------------------------------------------------------
# Entry point
# --------------------------------------------------------------------------
def kernel(
    x,
    edge_index,
    edge_attr,
    W_l,
    b_l,
    W_r,
    b_r,
    W_e,
    att,
    bias,
    _return_extras=False,
    **run_kwargs,
):
    from concourse import bass_utils

    x = np.asarray(x, dtype=np.float32)
    edge_index = np.asarray(edge_index)
    edge_attr = np.asarray(edge_attr, dtype=np.float32)

    sched, in_maps = _host_prep(
        x, edge_index, edge_attr, W_l, b_l, W_r, b_r, W_e, att, bias
    )

    key = (sched["T"], tuple(sched["kA"]), tuple(sched["kB"]))
    if key not in _prog_cache:
        _prog_cache[key] = _build_program(sched)
    nc = _prog_cache[key]

    res = bass_utils.run_bass_kernel_spmd(
        nc, in_maps, core_ids=list(range(NCORES)), **run_kwargs
    )
    out = np.empty((N, OUT_C), dtype=np.float32)
    for k in range(NCORES):
        out[k * NODES_PER_CORE : (k + 1) * NODES_PER_CORE] = res.results[k]["out"][
            :NODES_PER_CORE
        ]
    if _return_extras:
        return out, res
    return out
